# revision 19
# baseline (speedup 1.0000x reference)
"""Trainium2 Bass kernel for nn_Decoder_Select (MoE-routed dual decoder).

Strategy
--------
- 8 (b, s) examples -> one NeuronCore each (data-parallel over batch).
- Routing (ground_truth) is resolved on the HOST: each core receives the
  weights of its *selected* decoder as inputs, so the SPMD program is
  uniform (3 sources; decoder-0 weights are zero-padded to 3 sources and
  the dead source is zeroed on the host after gather).
- Algebraic restructuring (all linear 1x1 convs commute with the
  positional `fold`):
    reference: prelu -> fo(1x1) -> fold -> no/ng(1x1) -> tanh*sigm
               -> mn(1x1) -> relu -> *mixture -> ConvT1d
    kernel:    prelu -> fold -> A/B(1x1) -> tanh*sigm -> mn -> relu*mix
               -> ConvT1d
  where A_s = no_w @ fo_w_s, B_s = ng_w @ fo_w_s are composed on host, and
  biases fold to bu_s = 2*no_w@fo_b_s + no_b, bg_s = 2*ng_w@fo_b_s + ng_b
  (every folded position is covered by exactly 2 chunks).
- fold is a strided add: xf[c, 50m+j] = xp[c,j,m+2] + xp[c,j+50,m+1],
  split between DVE (early blocks) and GPSIMD (late blocks).
- ConvT1d(stride 8, ks 16): wav[8f+r] = sum_c src[c,f] P[c,r]
  + sum_c src[c,f-1] Q[c,r], done as two accumulating matmuls per
  128-frame tile with src as the stationary operand: the second uses a
  one-frame-shifted lhsT window (src tiles carry a leading zero column),
  49 tiles pack into one PSUM bank (8 cols each) -> one ACT copy/source.
- Selector pooling runs on the PE: s1_w @ x accumulated across column
  blocks into one PSUM bank, then a single small reduction.
- Sources 0/1 share the mask stage: their mn outputs land at PSUM
  partition offsets 0/64 of one bank, one fused relu*mixture DVE op.
- bf16 matmul datapath (PSUM accumulation and activations in fp32).
"""

import sys

sys.path.insert(0, "/opt/trn_rl_repo")

import ml_dtypes
import numpy as np

import concourse.bacc as bacc
import concourse.bass as bass
import concourse.tile as tile
from concourse.tile import add_dep_helper
from concourse import mybir
from concourse.bass_utils import run_bass_kernel_spmd

# Problem constants
BN = 128          # bottleneck channels
CK = 100          # chunk size
K = 127           # n chunks
FX = CK * K       # 12700 flat free size of x
NF = 6200         # folded frames
IC = 64           # IN_CHAN
NSRC = 3          # uniform padded source count
T = 8 * (NF + 1)  # 49608 output samples
B, S = 4, 2
N_SRCS = (2, 3)

F = 500           # free-dim block size (psum <= 512 fp32)
BLOCKS = [(i * F, min(F, NF - i * F)) for i in range((NF + F - 1) // F)]  # 12x500+200
NCHUNK = 4
CHW = FX // NCHUNK   # 3175
SELN = 454           # selector matmul column block (7 per 3175 chunk)
NFT = 49             # ConvT 128-frame tiles (49*128 = 6272 >= 6201)
FPAD = NFT * 128     # 6272 padded frames
NDVE_FOLD = 7        # fold blocks on DVE; rest on GPSIMD

# packed-constant column layout (bf16 pack: PH, fp32 pack: PF)
PH_WA = 0            # [0:384)   wA lhsT, (c, s*128+o)
PH_WB = 384          # [384:768) wB lhsT
PH_MN = 768          # [768:832) mn lhsT (c, o)
PH_TC = 832          # [832:848) tc lhsT on partitions 0:64
PH_S1 = 848          # [848:912) ws1 lhsT (bf16)
PH_W = 912
PF_AL = 0            # alpha broadcast to all partitions
PF_BU = 1            # [1:4)
PF_BG = 4            # [4:7)
PF_B1 = 7            # bs1 on partitions 0:64
PF_S2 = 8            # [8:10) ws2 lhsT on partitions 0:64
PF_B2 = 10           # bs2 on partitions 0:2
PF_W = 11

FP = mybir.dt.float32
BF = mybir.dt.bfloat16


def _build_program():
    nc = bacc.Bacc("TRN2", target_bir_lowering=False, debug=False, num_devices=8)

    x_d = nc.dram_tensor("x", [BN, FX], BF, kind="ExternalInput")
    mw_d = nc.dram_tensor("mw2", [BN, NF], BF, kind="ExternalInput")
    wph_d = nc.dram_tensor("wph", [BN, PH_W], BF, kind="ExternalInput")
    wpf_d = nc.dram_tensor("wpf", [BN, PF_W], FP, kind="ExternalInput")

    wav_d = nc.dram_tensor("wav8", [BN, NSRC, 8 * NFT], FP, kind="ExternalOutput")
    sel_d = nc.dram_tensor("sel", [2, 1], FP, kind="ExternalOutput")

    AF = mybir.ActivationFunctionType
    OP = mybir.AluOpType

    with tile.TileContext(nc) as tc:
        with (
            tc.tile_pool(name="consts", bufs=1) as consts,
            tc.tile_pool(name="xchunk", bufs=4) as xchunk,
            tc.tile_pool(name="xpp", bufs=1) as xpp,
            tc.tile_pool(name="xfp", bufs=1) as xfp,
            tc.tile_pool(name="srcp", bufs=1) as srcp,
            tc.tile_pool(name="ublk", bufs=3) as ublkp,
            tc.tile_pool(name="gblk", bufs=3) as gblkp,
            tc.tile_pool(name="y2blk", bufs=4) as y2blkp,
            tc.tile_pool(name="wav8", bufs=1) as wav8p,
            tc.tile_pool(name="small", bufs=1) as small,
            tc.tile_pool(name="psu", bufs=3, space="PSUM") as psu,
            tc.tile_pool(name="psg", bufs=2, space="PSUM") as psg,
            tc.tile_pool(name="psm", bufs=2, space="PSUM") as psm,
            tc.tile_pool(name="psw", bufs=1, space="PSUM") as psw,
        ):
            # ---- constant loads (gpsimd SWDGE; sync/scalar kept for x) ----------
            wpf = consts.tile([BN, PF_W], FP)
            nc.sync.dma_start(out=wpf[:], in_=wpf_d.ap()[:, :])
            wph = consts.tile([BN, PH_W], BF)
            nc.gpsimd.dma_start(out=wph[:], in_=wph_d.ap()[:, :])
            mw2_sb = consts.tile([BN, NF], BF)
            mw_dma = nc.gpsimd.dma_start(out=mw2_sb[:], in_=mw_d.ap()[:, :])

            wA = lambda s: wph[:, PH_WA + s * BN : PH_WA + (s + 1) * BN]
            wB = lambda s: wph[:, PH_WB + s * BN : PH_WB + (s + 1) * BN]
            wmn = wph[:, PH_MN : PH_MN + IC]
            wtcP = [wph[0:IC, PH_TC : PH_TC + 8], wph[IC:BN, PH_TC : PH_TC + 8]]
            wtcQ = [
                wph[0:IC, PH_TC + 8 : PH_TC + 16],
                wph[IC:BN, PH_TC + 8 : PH_TC + 16],
            ]
            ws1 = wph[:, PH_S1 : PH_S1 + IC]
            al_sb = wpf[:, PF_AL : PF_AL + 1]
            bu = lambda s: wpf[:, PF_BU + s : PF_BU + s + 1]
            bg = lambda s: wpf[:, PF_BG + s : PF_BG + s + 1]
            bs1 = wpf[0:IC, PF_B1 : PF_B1 + 1]
            ws2 = wpf[0:IC, PF_S2 : PF_S2 + 2]
            bs2 = wpf[0:2, PF_B2 : PF_B2 + 1]

            # ---- load x, PReLU (ACT), selector partial sums (PE) ----------------
            xp = xpp.tile([BN, FX], FP)
            ps_hacc = psm.tile([IC, 512], FP, tag="pm")
            xcs = []
            prev_dma = None
            for c in range(NCHUNK):
                xc = xchunk.tile([BN, CHW], BF, tag="xc", name=f"xc{c}")
                xcs.append(xc)
                d = (nc.sync if c % 2 == 0 else nc.scalar).dma_start(
                    out=xc[:], in_=x_d.ap()[:, c * CHW : (c + 1) * CHW]
                )
                if prev_dma is not None:
                    # serialize transfers in consumption order: the DMA engines
                    # round-robin concurrent transfers, which would delay chunk 0
                    # to the end of the whole load otherwise
                    add_dep_helper(d.ins, prev_dma.ins, reason="x chunk order")
                prev_dma = d
                if c == 1:
                    add_dep_helper(mw_dma.ins, d.ins, reason="mw after x c1")
                if c < 2:
                    nc.scalar.activation(
                        out=xp[:, c * CHW : (c + 1) * CHW],
                        in_=xc[:],
                        func=AF.Prelu,
                        alpha=al_sb,
                    )
                else:
                    # prelu(x) = max(alpha*x, x) for alpha in [0, 1]
                    nc.vector.scalar_tensor_tensor(
                        out=xp[:, c * CHW : (c + 1) * CHW],
                        in0=xc[:],
                        scalar=al_sb,
                        in1=xc[:],
                        op0=OP.mult,
                        op1=OP.max,
                    )
            first = True
            for c in range(NCHUNK):
                for j0 in range(0, CHW, SELN):
                    jw = min(SELN, CHW - j0)
                    nc.tensor.matmul(
                        ps_hacc[:, 0:jw], ws1, xcs[c][:, j0 : j0 + jw],
                        start=first, stop=(c == NCHUNK - 1 and j0 + SELN >= CHW),
                        skip_group_check=True,
                    )
                    first = False

            # ---- fold: xf[c, 50m+j] = xp[c, j, m+2] + xp[c, j+50, m+1] ----------
            xp3 = xp[:].rearrange("p (i k) -> p i k", k=K)
            xf = xfp.tile([BN, NF], BF)
            for bi, (f0, fw) in enumerate(BLOCKS):
                m0, mt = f0 // 50, fw // 50
                a_v = xp3[:, 0:50, m0 + 2 : m0 + 2 + mt].rearrange("p j m -> p m j")
                b_v = xp3[:, 50:100, m0 + 1 : m0 + 1 + mt].rearrange("p j m -> p m j")
                o_v = xf[:, f0 : f0 + fw].rearrange("p (m j) -> p m j", j=50)
                nc.vector.tensor_tensor(out=o_v, in0=a_v, in1=b_v, op=OP.add)

            # ---- decoder pipelines ----------------------------------------------
            # src tiles carry a leading zero column (frame -1) for the ConvT
            # shifted window.
            src_sb = []
            for s in range(NSRC):
                t_ = srcp.tile([IC, 1 + FPAD], BF, tag=f"src{s}", name=f"src{s}")
                nc.vector.memset(t_[:, 0:1], 0.0)
                nc.vector.memset(t_[:, 1 + NF :], 0.0)
                src_sb.append(t_)

            def ug_stage(s, f0, fw):
                ps_u = psu.tile([BN, F], FP, tag="pu", name=f"psu{s}")
                nc.tensor.matmul(
                    ps_u[:, 0:fw], wA(s), xf[:, f0 : f0 + fw],
                    start=True, stop=True,
                )
                u_b = ublkp.tile([BN, F], BF, tag="ub", name=f"ub{s}")
                nc.scalar.activation(
                    out=u_b[:, 0:fw], in_=ps_u[:, 0:fw], func=AF.Tanh, bias=bu(s)
                )
                ps_g = psg.tile([BN, F], FP, tag="pg", name=f"psg{s}")
                nc.tensor.matmul(
                    ps_g[:, 0:fw], wB(s), xf[:, f0 : f0 + fw],
                    start=True, stop=True,
                )
                g_b = gblkp.tile([BN, F], BF, tag="gb", name=f"gb{s}")
                nc.scalar.activation(
                    out=g_b[:, 0:fw], in_=ps_g[:, 0:fw], func=AF.Sigmoid, bias=bg(s)
                )
                y2_b = y2blkp.tile([BN, F], BF, tag="yb", name=f"yb{s}")
                nc.vector.tensor_tensor(
                    out=y2_b[:, 0:fw], in0=u_b[:, 0:fw], in1=g_b[:, 0:fw],
                    op=OP.mult,
                )
                return y2_b

            for f0, fw in BLOCKS:
                for s in range(NSRC):
                    y_b = ug_stage(s, f0, fw)
                    ps_m = psm.tile([IC, F], FP, tag="pm", name=f"psm{s}")
                    nc.tensor.matmul(
                        ps_m[:, 0:fw], wmn, y_b[:, 0:fw], start=True, stop=True
                    )
                    last_mask = nc.vector.scalar_tensor_tensor(
                        out=src_sb[s][:, 1 + f0 : 1 + f0 + fw],
                        in0=ps_m[:, 0:fw],
                        scalar=0.0,
                        in1=mw2_sb[0:IC, f0 : f0 + fw],
                        op0=OP.max,
                        op1=OP.mult,
                    )


            # ---- selector epilogue (emitted late: keeps Scalar FIFO clear) ------
            hsum = small.tile([IC, 1], FP)
            red = nc.vector.tensor_reduce(
                out=hsum[:], in_=ps_hacc[0:IC, 0:SELN],
                axis=mybir.AxisListType.X, op=OP.add,
            )
            add_dep_helper(red.ins, last_mask.ins, reason="defer selector to tail")
            h_sb = small.tile([IC, 1], FP)
            nc.scalar.activation(
                out=h_sb[:], in_=hsum[:], func=AF.Relu, bias=bs1,
                scale=1.0 / (CK * K),
            )
            ps_sel = psm.tile([2, 1], FP, tag="pm")
            nc.tensor.matmul(ps_sel[:], ws2, h_sb[:], start=True, stop=True)
            sel_sb = small.tile([2, 1], FP)
            nc.scalar.activation(
                out=sel_sb[:], in_=ps_sel[:], func=AF.Identity, bias=bs2
            )
            nc.sync.dma_start(out=sel_d.ap()[:, :], in_=sel_sb[:])

            # ---- ConvT1d: wav[p, 8t+r] = Z[128t+p, r] + Z[128t+p-1, r+8] --------
            w8all = wav8p.tile([BN, NSRC, 8 * NFT], FP, tag="w8")
            for s in range(NSRC):
                sv = src_sb[s]
                ps_w = psw.tile([BN, 8 * NFT], FP, tag="pw", name=f"psw{s}")
                for t in range(NFT):
                    nc.tensor.matmul(
                        ps_w[:, 8 * t : 8 * t + 8],
                        sv[:, 1 + 128 * t : 1 + 128 * (t + 1)],
                        wtcP[0],
                        start=(t == 0), stop=False, skip_group_check=True,
                    )
                for t in range(NFT):
                    nc.tensor.matmul(
                        ps_w[:, 8 * t : 8 * t + 8],
                        sv[:, 128 * t : 128 * (t + 1)],
                        wtcQ[0],
                        start=False, stop=(t == NFT - 1), skip_group_check=True,
                    )
                nc.vector.tensor_copy(out=w8all[:, s, :], in_=ps_w[:])
            nc.sync.dma_start(out=wav_d.ap()[:, :, :], in_=w8all[:])

    nc.compile()
    return nc


_PROG = {}


def _get_program():
    if "nc" not in _PROG:
        _PROG["nc"] = _build_program()
    return _PROG["nc"]


def _make_in_maps(inputs):
    f32 = lambda a: np.ascontiguousarray(np.asarray(a), dtype=np.float32)
    bf16 = ml_dtypes.bfloat16
    outputs = f32(inputs["outputs"])          # (4, 2, 128, 100, 127)
    mixture_w = f32(inputs["mixture_w"])      # (4, 64, 6200)
    gt = np.asarray(inputs["ground_truth"]).astype(np.int64)  # (4,)

    ws1 = f32(inputs["s1_w"]).T               # (128, 64)
    bs1 = f32(inputs["s1_b"])
    ws2 = f32(inputs["s2_w"]).T               # (64, 2)
    bs2 = f32(inputs["s2_b"])

    dec = {}
    for d in (0, 1):
        sfx = str(d)
        n_src = N_SRCS[d]
        fo_w = f32(inputs[f"fo{sfx}_w"]).astype(np.float64)  # (n_src*128, 128)
        fo_b = f32(inputs[f"fo{sfx}_b"]).astype(np.float64)
        no_w = f32(inputs[f"no{sfx}_w"]).astype(np.float64)
        no_b = f32(inputs[f"no{sfx}_b"]).astype(np.float64)
        ng_w = f32(inputs[f"ng{sfx}_w"]).astype(np.float64)
        ng_b = f32(inputs[f"ng{sfx}_b"]).astype(np.float64)
        mn_w = f32(inputs[f"mn{sfx}_w"])                     # (64, 128)
        tc_w = f32(inputs[f"tc{sfx}_w"])[:, 0, :]            # (64, 16)
        a = float(np.asarray(inputs[f"a{sfx}"]).reshape(-1)[0])

        wA = np.zeros((NSRC, BN, BN), np.float32)
        wB = np.zeros((NSRC, BN, BN), np.float32)
        bu = np.zeros((NSRC, BN), np.float32)
        bg = np.zeros((NSRC, BN), np.float32)
        for s in range(n_src):
            fws = fo_w[s * BN : (s + 1) * BN]                # (128, 128)
            fbs = fo_b[s * BN : (s + 1) * BN]
            wA[s] = (no_w @ fws).T.astype(np.float32)
            wB[s] = (ng_w @ fws).T.astype(np.float32)
            bu[s] = (2.0 * (no_w @ fbs) + no_b).astype(np.float32)
            bg[s] = (2.0 * (ng_w @ fbs) + ng_b).astype(np.float32)
        for s in range(n_src, NSRC):
            bu[s] = no_b.astype(np.float32)
            bg[s] = ng_b.astype(np.float32)

        wph = np.zeros((BN, PH_W), bf16)
        wph[:, PH_WA : PH_WA + 384] = wA.transpose(1, 0, 2).reshape(BN, 384)
        wph[:, PH_WB : PH_WB + 384] = wB.transpose(1, 0, 2).reshape(BN, 384)
        wph[:, PH_MN : PH_MN + IC] = mn_w.T
        wph[0:IC, PH_TC : PH_TC + 16] = tc_w
        wph[IC:BN, PH_TC : PH_TC + 16] = tc_w
        wph[:, PH_S1 : PH_S1 + IC] = ws1

        wpf = np.zeros((BN, PF_W), np.float32)
        wpf[:, PF_AL] = a
        wpf[:, PF_BU : PF_BU + NSRC] = bu.T
        wpf[:, PF_BG : PF_BG + NSRC] = bg.T
        wpf[0:IC, PF_B1] = bs1
        wpf[0:IC, PF_S2 : PF_S2 + 2] = ws2
        wpf[0:2, PF_B2] = bs2
        dec[d] = (wph, wpf)

    in_maps = []
    for b in range(B):
        wph, wpf = dec[int(gt[b] != 0)]
        mwb = np.zeros((BN, NF), bf16)
        mwb[0:IC] = mixture_w[b].astype(bf16)
        mwb[IC:BN] = mwb[0:IC]
        for s in range(S):
            in_maps.append(
                dict(
                    x=np.ascontiguousarray(
                        outputs[b, s].reshape(BN, FX).astype(bf16)
                    ),
                    mw2=mwb,
                    wph=wph,
                    wpf=wpf,
                )
            )
    return in_maps, gt


def _assemble(results, gt):
    wavs = np.zeros((B, S, NSRC, T), np.float32)
    sel = np.zeros((B, S, 2), np.float32)
    for b in range(B):
        for s in range(S):
            r = results[b * S + s]
            w8 = r["wav8"]                    # (128, 3, 8*NFT)
            full = (
                w8.reshape(BN, NSRC, NFT, 8)
                .transpose(1, 2, 0, 3)
                .reshape(NSRC, -1)
            )
            wavs[b, s] = full[:, :T]
            if gt[b] == 0:
                wavs[b, s, N_SRCS[0] :] = 0.0
            sel[b, s] = r["sel"][:, 0]
    return wavs, sel


def run(inputs, trace=False):
    nc = _get_program()
    in_maps, gt = _make_in_maps(inputs)
    res = run_bass_kernel_spmd(nc, in_maps, core_ids=list(range(8)), trace=trace)
    wavs, sel = _assemble(res.results, gt)
    return (wavs, sel), res


def kernel(**inputs):
    (wavs, sel), _ = run(inputs, trace=False)
    return wavs, sel


# revision 20
# speedup vs baseline: 1.1280x; 1.1280x over previous
"""Trainium2 Bass kernel for nn_Decoder_Select (MoE-routed dual decoder).

Strategy
--------
- 8 (b, s) examples -> one NeuronCore each (data-parallel over batch).
- Routing (ground_truth) is resolved on the HOST: each core receives the
  weights of its *selected* decoder as inputs, so the SPMD program is
  uniform (3 sources; decoder-0 weights are zero-padded to 3 sources and
  the dead source is zeroed on the host after gather).
- Algebraic restructuring (all linear 1x1 convs commute with the
  positional `fold`):
    reference: prelu -> fo(1x1) -> fold -> no/ng(1x1) -> tanh*sigm
               -> mn(1x1) -> relu -> *mixture -> ConvT1d
    kernel:    prelu -> fold -> A/B(1x1) -> tanh*sigm -> mn -> relu*mix
               -> ConvT1d
  where A_s = no_w @ fo_w_s, B_s = ng_w @ fo_w_s are composed on host, and
  biases fold to bu_s = 2*no_w@fo_b_s + no_b, bg_s = 2*ng_w@fo_b_s + ng_b
  (every folded position is covered by exactly 2 chunks).
- fold is a strided add: xf[c, 50m+j] = xp[c,j,m+2] + xp[c,j+50,m+1],
  split between DVE (early blocks) and GPSIMD (late blocks).
- ConvT1d(stride 8, ks 16): wav[8f+r] = sum_c src[c,f] P[c,r]
  + sum_c src[c,f-1] Q[c,r], done as two accumulating matmuls per
  128-frame tile with src as the stationary operand: the second uses a
  one-frame-shifted lhsT window (src tiles carry a leading zero column),
  49 tiles pack into one PSUM bank (8 cols each) -> one ACT copy/source.
- Selector pooling runs on the PE: s1_w @ x accumulated across column
  blocks into one PSUM bank, then a single small reduction.
- Sources 0/1 share the mask stage: their mn outputs land at PSUM
  partition offsets 0/64 of one bank, one fused relu*mixture DVE op.
- bf16 matmul datapath (PSUM accumulation and activations in fp32).
"""

import sys

sys.path.insert(0, "/opt/trn_rl_repo")

import ml_dtypes
import numpy as np

import concourse.bacc as bacc
import concourse.bass as bass
import concourse.tile as tile
from concourse.tile import add_dep_helper
from concourse import mybir
from concourse.bass_utils import run_bass_kernel_spmd

# Problem constants
BN = 128          # bottleneck channels
CK = 100          # chunk size
K = 127           # n chunks
FX = CK * K       # 12700 flat free size of x
NF = 6200         # folded frames
IC = 64           # IN_CHAN
NSRC = 3          # uniform padded source count
T = 8 * (NF + 1)  # 49608 output samples
B, S = 4, 2
N_SRCS = (2, 3)

F = 500           # free-dim block size (psum <= 512 fp32)
BLOCKS = [(i * F, min(F, NF - i * F)) for i in range((NF + F - 1) // F)]  # 12x500+200
NCHUNK = 4
CHW = FX // NCHUNK   # 3175
SELN = 454           # selector matmul column block (7 per 3175 chunk)
NFT = 49             # ConvT 128-frame tiles (49*128 = 6272 >= 6201)
FPAD = NFT * 128     # 6272 padded frames
NDVE_FOLD = 7        # fold blocks on DVE; rest on GPSIMD

# packed-constant column layout (bf16 pack: PH, fp32 pack: PF)
PH_WA = 0            # [0:384)   wA lhsT, (c, s*128+o)
PH_WB = 384          # [384:768) wB lhsT
PH_MN = 768          # [768:832) mn lhsT (c, o)
PH_TC = 832          # [832:848) tc lhsT on partitions 0:64
PH_S1 = 848          # [848:912) ws1 lhsT (bf16)
PH_W = 912
PF_AL = 0            # alpha broadcast to all partitions
PF_BU = 1            # [1:4)
PF_BG = 4            # [4:7)
PF_B1 = 7            # bs1 on partitions 0:64
PF_S2 = 8            # [8:10) ws2 lhsT on partitions 0:64
PF_B2 = 10           # bs2 on partitions 0:2
PF_W = 11

FP = mybir.dt.float32
BF = mybir.dt.bfloat16


def _build_program():
    nc = bacc.Bacc("TRN2", target_bir_lowering=False, debug=False, num_devices=8)

    x_d = nc.dram_tensor("x", [BN, FX], BF, kind="ExternalInput")
    mw_d = nc.dram_tensor("mw2", [BN, NF], BF, kind="ExternalInput")
    wph_d = nc.dram_tensor("wph", [BN, PH_W], BF, kind="ExternalInput")
    wpf_d = nc.dram_tensor("wpf", [BN, PF_W], FP, kind="ExternalInput")

    wav_d = nc.dram_tensor("wav8", [BN, NSRC, 8 * NFT], FP, kind="ExternalOutput")
    sel_d = nc.dram_tensor("sel", [2, 1], FP, kind="ExternalOutput")

    AF = mybir.ActivationFunctionType
    OP = mybir.AluOpType

    with tile.TileContext(nc) as tc:
        with (
            tc.tile_pool(name="consts", bufs=1) as consts,
            tc.tile_pool(name="xchunk", bufs=4) as xchunk,
            tc.tile_pool(name="xpp", bufs=1) as xpp,
            tc.tile_pool(name="xfp", bufs=1) as xfp,
            tc.tile_pool(name="srcp", bufs=1) as srcp,
            tc.tile_pool(name="ublk", bufs=3) as ublkp,
            tc.tile_pool(name="gblk", bufs=3) as gblkp,
            tc.tile_pool(name="y2blk", bufs=4) as y2blkp,
            tc.tile_pool(name="wav8", bufs=1) as wav8p,
            tc.tile_pool(name="small", bufs=1) as small,
            tc.tile_pool(name="psu", bufs=3, space="PSUM") as psu,
            tc.tile_pool(name="psg", bufs=2, space="PSUM") as psg,
            tc.tile_pool(name="psm", bufs=2, space="PSUM") as psm,
            tc.tile_pool(name="psw", bufs=1, space="PSUM") as psw,
        ):
            # ---- constant loads (gpsimd SWDGE; sync/scalar kept for x) ----------
            wpf = consts.tile([BN, PF_W], FP)
            nc.scalar.dma_start(out=wpf[:], in_=wpf_d.ap()[:, :])
            wph = consts.tile([BN, PH_W], BF)
            nc.gpsimd.dma_start(out=wph[:], in_=wph_d.ap()[:, :])
            mw2_sb = consts.tile([BN, NF], BF)
            mw_dma = nc.gpsimd.dma_start(out=mw2_sb[:], in_=mw_d.ap()[:, :])

            wA = lambda s: wph[:, PH_WA + s * BN : PH_WA + (s + 1) * BN]
            wB = lambda s: wph[:, PH_WB + s * BN : PH_WB + (s + 1) * BN]
            wmn = wph[:, PH_MN : PH_MN + IC]
            wtcP = [wph[0:IC, PH_TC : PH_TC + 8], wph[IC:BN, PH_TC : PH_TC + 8]]
            wtcQ = [
                wph[0:IC, PH_TC + 8 : PH_TC + 16],
                wph[IC:BN, PH_TC + 8 : PH_TC + 16],
            ]
            ws1 = wph[:, PH_S1 : PH_S1 + IC]
            al_sb = wpf[:, PF_AL : PF_AL + 1]
            bu = lambda s: wpf[:, PF_BU + s : PF_BU + s + 1]
            bg = lambda s: wpf[:, PF_BG + s : PF_BG + s + 1]
            bs1 = wpf[0:IC, PF_B1 : PF_B1 + 1]
            ws2 = wpf[0:IC, PF_S2 : PF_S2 + 2]
            bs2 = wpf[0:2, PF_B2 : PF_B2 + 1]

            # ---- load x, PReLU (ACT), selector partial sums (PE) ----------------
            xp = xpp.tile([BN, FX], FP)
            ps_hacc = psm.tile([IC, 512], FP, tag="pm")
            xcs = []
            for c in range(NCHUNK):
                xc = xchunk.tile([BN, CHW], BF, tag="xc", name=f"xc{c}")
                xcs.append(xc)
                # all on the sync HWDGE ring: per-ring FIFO serializes the
                # transfers in consumption order with pipelined issue
                d = nc.sync.dma_start(
                    out=xc[:], in_=x_d.ap()[:, c * CHW : (c + 1) * CHW]
                )
                if c == NCHUNK - 1:
                    add_dep_helper(mw_dma.ins, d.ins, reason="mw after x")
                nc.scalar.activation(
                    out=xp[:, c * CHW : (c + 1) * CHW],
                    in_=xc[:],
                    func=AF.Prelu,
                    alpha=al_sb,
                )
            first = True
            for c in range(NCHUNK):
                for j0 in range(0, CHW, SELN):
                    jw = min(SELN, CHW - j0)
                    nc.tensor.matmul(
                        ps_hacc[:, 0:jw], ws1, xcs[c][:, j0 : j0 + jw],
                        start=first, stop=(c == NCHUNK - 1 and j0 + SELN >= CHW),
                        skip_group_check=True,
                    )
                    first = False

            # ---- fold: xf[c, 50m+j] = xp[c, j, m+2] + xp[c, j+50, m+1] ----------
            xp3 = xp[:].rearrange("p (i k) -> p i k", k=K)
            xf = xfp.tile([BN, NF], BF)
            for bi, (f0, fw) in enumerate(BLOCKS):
                m0, mt = f0 // 50, fw // 50
                a_v = xp3[:, 0:50, m0 + 2 : m0 + 2 + mt].rearrange("p j m -> p m j")
                b_v = xp3[:, 50:100, m0 + 1 : m0 + 1 + mt].rearrange("p j m -> p m j")
                o_v = xf[:, f0 : f0 + fw].rearrange("p (m j) -> p m j", j=50)
                nc.vector.tensor_tensor(out=o_v, in0=a_v, in1=b_v, op=OP.add)

            # ---- decoder pipelines ----------------------------------------------
            # src tiles carry a leading zero column (frame -1) for the ConvT
            # shifted window.
            src_sb = []
            for s in range(NSRC):
                t_ = srcp.tile([IC, 1 + FPAD], BF, tag=f"src{s}", name=f"src{s}")
                nc.vector.memset(t_[:, 0:1], 0.0)
                nc.vector.memset(t_[:, 1 + NF :], 0.0)
                src_sb.append(t_)

            def ug_stage(s, f0, fw):
                ps_u = psu.tile([BN, F], FP, tag="pu", name=f"psu{s}")
                nc.tensor.matmul(
                    ps_u[:, 0:fw], wA(s), xf[:, f0 : f0 + fw],
                    start=True, stop=True,
                )
                u_b = ublkp.tile([BN, F], BF, tag="ub", name=f"ub{s}")
                nc.scalar.activation(
                    out=u_b[:, 0:fw], in_=ps_u[:, 0:fw], func=AF.Tanh, bias=bu(s)
                )
                ps_g = psg.tile([BN, F], FP, tag="pg", name=f"psg{s}")
                nc.tensor.matmul(
                    ps_g[:, 0:fw], wB(s), xf[:, f0 : f0 + fw],
                    start=True, stop=True,
                )
                g_b = gblkp.tile([BN, F], BF, tag="gb", name=f"gb{s}")
                nc.scalar.activation(
                    out=g_b[:, 0:fw], in_=ps_g[:, 0:fw], func=AF.Sigmoid, bias=bg(s)
                )
                y2_b = y2blkp.tile([BN, F], BF, tag="yb", name=f"yb{s}")
                nc.vector.tensor_tensor(
                    out=y2_b[:, 0:fw], in0=u_b[:, 0:fw], in1=g_b[:, 0:fw],
                    op=OP.mult,
                )
                return y2_b

            for f0, fw in BLOCKS:
                for s in range(NSRC):
                    y_b = ug_stage(s, f0, fw)
                    ps_m = psm.tile([IC, F], FP, tag="pm", name=f"psm{s}")
                    nc.tensor.matmul(
                        ps_m[:, 0:fw], wmn, y_b[:, 0:fw], start=True, stop=True
                    )
                    mask_inst = nc.vector.scalar_tensor_tensor(
                        out=src_sb[s][:, 1 + f0 : 1 + f0 + fw],
                        in0=ps_m[:, 0:fw],
                        scalar=0.0,
                        in1=mw2_sb[0:IC, f0 : f0 + fw],
                        op0=OP.max,
                        op1=OP.mult,
                    )
                    if f0 == 6 * F and s == 2:
                        mid_mask = mask_inst


            # ---- selector epilogue (emitted late: keeps Scalar FIFO clear) ------
            hsum = small.tile([IC, 1], FP)
            red = nc.vector.tensor_reduce(
                out=hsum[:], in_=ps_hacc[0:IC, 0:SELN],
                axis=mybir.AxisListType.X, op=OP.add,
            )
            add_dep_helper(red.ins, mid_mask.ins, reason="defer selector to mid-stream")
            h_sb = small.tile([IC, 1], FP)
            nc.scalar.activation(
                out=h_sb[:], in_=hsum[:], func=AF.Relu, bias=bs1,
                scale=1.0 / (CK * K),
            )
            ps_sel = psm.tile([2, 1], FP, tag="pm")
            nc.tensor.matmul(ps_sel[:], ws2, h_sb[:], start=True, stop=True)
            sel_sb = small.tile([2, 1], FP)
            nc.scalar.activation(
                out=sel_sb[:], in_=ps_sel[:], func=AF.Identity, bias=bs2
            )
            nc.sync.dma_start(out=sel_d.ap()[:, :], in_=sel_sb[:])

            # ---- ConvT1d: wav[p, 8t+r] = Z[128t+p, r] + Z[128t+p-1, r+8] --------
            w8all = wav8p.tile([BN, NSRC, 8 * NFT], FP, tag="w8")
            for s in range(NSRC):
                sv = src_sb[s]
                ps_w = psw.tile([BN, 8 * NFT], FP, tag="pw", name=f"psw{s}")
                for t in range(NFT):
                    nc.tensor.matmul(
                        ps_w[:, 8 * t : 8 * t + 8],
                        sv[:, 1 + 128 * t : 1 + 128 * (t + 1)],
                        wtcP[0],
                        start=(t == 0), stop=False, skip_group_check=True,
                    )
                for t in range(NFT):
                    nc.tensor.matmul(
                        ps_w[:, 8 * t : 8 * t + 8],
                        sv[:, 128 * t : 128 * (t + 1)],
                        wtcQ[0],
                        start=False, stop=(t == NFT - 1), skip_group_check=True,
                    )
                nc.vector.tensor_copy(out=w8all[:, s, :], in_=ps_w[:])
            nc.sync.dma_start(out=wav_d.ap()[:, :, :], in_=w8all[:])

    nc.compile()
    return nc


_PROG = {}


def _get_program():
    if "nc" not in _PROG:
        _PROG["nc"] = _build_program()
    return _PROG["nc"]


def _make_in_maps(inputs):
    f32 = lambda a: np.ascontiguousarray(np.asarray(a), dtype=np.float32)
    bf16 = ml_dtypes.bfloat16
    outputs = f32(inputs["outputs"])          # (4, 2, 128, 100, 127)
    mixture_w = f32(inputs["mixture_w"])      # (4, 64, 6200)
    gt = np.asarray(inputs["ground_truth"]).astype(np.int64)  # (4,)

    ws1 = f32(inputs["s1_w"]).T               # (128, 64)
    bs1 = f32(inputs["s1_b"])
    ws2 = f32(inputs["s2_w"]).T               # (64, 2)
    bs2 = f32(inputs["s2_b"])

    dec = {}
    for d in (0, 1):
        sfx = str(d)
        n_src = N_SRCS[d]
        fo_w = f32(inputs[f"fo{sfx}_w"]).astype(np.float64)  # (n_src*128, 128)
        fo_b = f32(inputs[f"fo{sfx}_b"]).astype(np.float64)
        no_w = f32(inputs[f"no{sfx}_w"]).astype(np.float64)
        no_b = f32(inputs[f"no{sfx}_b"]).astype(np.float64)
        ng_w = f32(inputs[f"ng{sfx}_w"]).astype(np.float64)
        ng_b = f32(inputs[f"ng{sfx}_b"]).astype(np.float64)
        mn_w = f32(inputs[f"mn{sfx}_w"])                     # (64, 128)
        tc_w = f32(inputs[f"tc{sfx}_w"])[:, 0, :]            # (64, 16)
        a = float(np.asarray(inputs[f"a{sfx}"]).reshape(-1)[0])

        wA = np.zeros((NSRC, BN, BN), np.float32)
        wB = np.zeros((NSRC, BN, BN), np.float32)
        bu = np.zeros((NSRC, BN), np.float32)
        bg = np.zeros((NSRC, BN), np.float32)
        for s in range(n_src):
            fws = fo_w[s * BN : (s + 1) * BN]                # (128, 128)
            fbs = fo_b[s * BN : (s + 1) * BN]
            wA[s] = (no_w @ fws).T.astype(np.float32)
            wB[s] = (ng_w @ fws).T.astype(np.float32)
            bu[s] = (2.0 * (no_w @ fbs) + no_b).astype(np.float32)
            bg[s] = (2.0 * (ng_w @ fbs) + ng_b).astype(np.float32)
        for s in range(n_src, NSRC):
            bu[s] = no_b.astype(np.float32)
            bg[s] = ng_b.astype(np.float32)

        wph = np.zeros((BN, PH_W), bf16)
        wph[:, PH_WA : PH_WA + 384] = wA.transpose(1, 0, 2).reshape(BN, 384)
        wph[:, PH_WB : PH_WB + 384] = wB.transpose(1, 0, 2).reshape(BN, 384)
        wph[:, PH_MN : PH_MN + IC] = mn_w.T
        wph[0:IC, PH_TC : PH_TC + 16] = tc_w
        wph[IC:BN, PH_TC : PH_TC + 16] = tc_w
        wph[:, PH_S1 : PH_S1 + IC] = ws1

        wpf = np.zeros((BN, PF_W), np.float32)
        wpf[:, PF_AL] = a
        wpf[:, PF_BU : PF_BU + NSRC] = bu.T
        wpf[:, PF_BG : PF_BG + NSRC] = bg.T
        wpf[0:IC, PF_B1] = bs1
        wpf[0:IC, PF_S2 : PF_S2 + 2] = ws2
        wpf[0:2, PF_B2] = bs2
        dec[d] = (wph, wpf)

    in_maps = []
    for b in range(B):
        wph, wpf = dec[int(gt[b] != 0)]
        mwb = np.zeros((BN, NF), bf16)
        mwb[0:IC] = mixture_w[b].astype(bf16)
        mwb[IC:BN] = mwb[0:IC]
        for s in range(S):
            in_maps.append(
                dict(
                    x=np.ascontiguousarray(
                        outputs[b, s].reshape(BN, FX).astype(bf16)
                    ),
                    mw2=mwb,
                    wph=wph,
                    wpf=wpf,
                )
            )
    return in_maps, gt


def _assemble(results, gt):
    wavs = np.zeros((B, S, NSRC, T), np.float32)
    sel = np.zeros((B, S, 2), np.float32)
    for b in range(B):
        for s in range(S):
            r = results[b * S + s]
            w8 = r["wav8"]                    # (128, 3, 8*NFT)
            full = (
                w8.reshape(BN, NSRC, NFT, 8)
                .transpose(1, 2, 0, 3)
                .reshape(NSRC, -1)
            )
            wavs[b, s] = full[:, :T]
            if gt[b] == 0:
                wavs[b, s, N_SRCS[0] :] = 0.0
            sel[b, s] = r["sel"][:, 0]
    return wavs, sel


def run(inputs, trace=False):
    nc = _get_program()
    in_maps, gt = _make_in_maps(inputs)
    res = run_bass_kernel_spmd(nc, in_maps, core_ids=list(range(8)), trace=trace)
    wavs, sel = _assemble(res.results, gt)
    return (wavs, sel), res


def kernel(**inputs):
    (wavs, sel), _ = run(inputs, trace=False)
    return wavs, sel
